# revision 20
# baseline (speedup 1.0000x reference)
"""Causal self-attention (B=4, T=2048, D=1024, H=16) on 8 Trainium2 cores.

Sharding: core m = (batch b=m//2, head-group g=m%2 of 8 heads).
Each core computes QKV for its 8 heads of its batch, full causal attention
(exact lower-triangular tile skip, identical tile structure on every core),
writes its attention output transposed (y^T, [512 ch, 2048 tok]) to DRAM,
AllGather(8) so every core holds all head-groups, then projects its own
1024-token slice with the full w_proj.  Per-core programs are identical
(SPMD); the only cross-core variation is input data plus two uint32 scalars
driving dynamic DMA offsets in the post-AllGather gather.

All matmul operands are float32r (FP22-read, 1-pass PE) for bf16-class
throughput at near-fp32 accuracy; accumulation is fp32 in PSUM.
"""

import numpy as np

import concourse.bass as bass
import concourse.mybir as mybir
import concourse.tile as tile
from concourse import bacc
from concourse.bass_utils import run_bass_kernel_spmd
from concourse.masks import make_identity

FP = mybir.dt.float32r
F32 = mybir.dt.float32

# Problem constants (per spec; hardcoded).
B, T, D, H = 4, 2048, 1024, 16
DH = 64                      # head dim
N_CORES = 8
HC = H // 2                  # heads per core = 8
HP = HC // 2                 # head pairs per core = 4
DK = D // 128                # model-dim contraction tiles = 8
TT = T // 128                # token tiles of 128 = 16
QB = T // 512                # q blocks of 512 = 4
TC = T // 512                # token chunks of 512 = 4
MYT = T // 2                 # tokens this core projects = 1024
SCALE = 1.0 / 8.0            # 1/sqrt(DH)
DEBUG = False                # add intermediate dumps as extra outputs
NO_COLLECTIVE = False        # replace AllGather with local copy (TimelineSim)


def build_kernel():
    nc = bacc.Bacc("TRN2", target_bir_lowering=False, debug=False,
                   num_devices=N_CORES)

    x_b = nc.dram_tensor("x_b", [T, D], FP, kind="ExternalInput").ap()
    w_qkv_my = nc.dram_tensor("w_qkv_my", [D, 3 * 512], FP,
                              kind="ExternalInput").ap()
    w_proj = nc.dram_tensor("w_proj", [D, D], FP, kind="ExternalInput").ap()
    sel = nc.dram_tensor("sel", [1, 2], mybir.dt.uint32,
                         kind="ExternalInput").ap()
    out = nc.dram_tensor("out", [MYT, D], F32, kind="ExternalOutput").ap()
    dbg = {}
    if DEBUG:
        dbg["qt"] = nc.dram_tensor("dbg_qt", [128, HP, T], F32,
                                   kind="ExternalOutput").ap()
        dbg["kt"] = nc.dram_tensor("dbg_kt", [128, HP, T], F32,
                                   kind="ExternalOutput").ap()
        dbg["v"] = nc.dram_tensor("dbg_v", [128, TT, HC, 65], F32,
                                  kind="ExternalOutput").ap()
        dbg["y"] = nc.dram_tensor("dbg_y", [128, HP, T], F32,
                                  kind="ExternalOutput").ap()
        dbg["yg"] = nc.dram_tensor("dbg_yg", [128, DK, MYT], F32,
                                   kind="ExternalOutput").ap()
        dbg["yall"] = nc.dram_tensor("dbg_yall", [N_CORES * HP, 128, T], F32,
                                     kind="ExternalOutput").ap()
        dbg["yloc"] = nc.dram_tensor("dbg_yloc", [HP, 128, T], F32,
                                     kind="ExternalOutput").ap()

    with tile.TileContext(nc) as tc:
        _emit(tc, x_b, w_qkv_my, w_proj, sel, out, dbg)

    nc.compile()
    return nc


def _emit(tc, x_b, w_qkv_my, w_proj, sel, out, dbg=None):
    from contextlib import ExitStack
    nc = tc.nc
    ctx = ExitStack()

    # ---- constant pools -------------------------------------------------
    const = ctx.enter_context(tc.tile_pool(name="const", bufs=1))
    ident_f32 = const.tile([128, 128], F32)
    make_identity(nc, ident_f32[:])
    ident = const.tile([128, 128], FP)
    nc.vector.tensor_copy(ident[:], ident_f32[:])

    # 4 diagonal-band masks [128 k, 512 q]: keep (1.0) where kk <= qq - 128*dk
    masks = []
    for dk in range(4):
        m = const.tile([128, 512], F32, tag=f"mask{dk}")
        nc.gpsimd.memset(m[:], 1.0)
        nc.gpsimd.affine_select(
            out=m[:], in_=m[:],
            compare_op=mybir.AluOpType.is_ge,
            fill=0.0,
            base=-128 * dk,
            pattern=[[1, 512]],       # + qq
            channel_multiplier=-1,    # - kk
        )
        masks.append(m)

    # ---- persistent SBUF tensors ---------------------------------------
    from contextlib import ExitStack as _ES
    persist = ctx.enter_context(tc.tile_pool(name="persist", bufs=1))
    # y^T for my 8 heads: [ch-within-pair, head-pair(=ctile), tokens]
    y_sb = persist.tile([128, HP, T], FP, tag="y")
    qkv_ctx = _ES()
    qkvp = qkv_ctx.enter_context(tc.tile_pool(name="qkvp", bufs=1))
    # Q^T / K^T: [d-within-pair (2x64), head-pair, tokens]
    qt_sb = qkvp.tile([128, HP, T], FP, tag="qt")
    kt_sb = qkvp.tile([128, HP, T], FP, tag="kt")
    # V augmented with a ones column per head: [k-part, ktile, head, 65]
    v_sb = qkvp.tile([128, TT, HC, 65], FP, tag="v")

    ones_f32 = const.tile([128, TT, HC, 1], F32)
    nc.vector.memset(ones_f32[:], 1.0)
    nc.vector.tensor_copy(v_sb[:, :, :, 64:65], ones_f32[:])  # ones columns

    # ---- phase A+B: x^T chunks and QKV ---------------------------------
    # w_qkv_my columns: [Q(512) | K(512) | V(512)], head h at 64h within each.
    w_re = w_qkv_my.rearrange("(o p) f -> p o f", p=128)
    with tc.tile_pool(name="wqkv", bufs=1) as wpool, \
         tc.tile_pool(name="wqk", bufs=3) as wqkp, \
         tc.tile_pool(name="xload", bufs=2) as xload, \
         tc.tile_pool(name="xt", bufs=2) as xtp, \
         tc.tile_pool(name="tpsum", bufs=2, space="PSUM") as tpsum, \
         tc.tile_pool(name="qkpsum", bufs=3, space="PSUM") as qkpsum:

        wv_sb = wpool.tile([128, DK, 512], FP)
        nc.sync.dma_start(wv_sb[:], w_re[:, :, 1024:1536])

        for tchunk in range(TC):
            # transpose x[tchunk*512 : +512, :] -> xT chunk [128, DK, 512]
            xt_chunk = xtp.tile([128, DK, 512], FP, tag="xtc")
            for tt in range(4):
                xrow = xload.tile([128, D], FP, tag="xrow")
                nc.sync.dma_start(
                    xrow[:], x_b[(tchunk * 4 + tt) * 128:(tchunk * 4 + tt + 1) * 128, :])
                for ct in range(DK):
                    ps = tpsum.tile([128, 128], FP, tag="tps")
                    nc.tensor.transpose(ps[:], xrow[:, ct * 128:(ct + 1) * 128],
                                        ident[:])
                    nc.scalar.copy(xt_chunk[:, ct, tt * 128:(tt + 1) * 128], ps[:])

            # Q^T and K^T for each head pair over this token chunk
            for hp in range(HP):
                for which, dst in ((0, qt_sb), (1, kt_sb)):
                    wt = wqkp.tile([128, DK, 128], FP, tag="wqk", name="wt")
                    nc.sync.dma_start(
                        wt[:], w_re[:, :, which * 512 + hp * 128:
                                    which * 512 + (hp + 1) * 128])
                    ps = qkpsum.tile([128, 512], F32, tag="qkps")
                    for kc in range(DK):
                        nc.tensor.matmul(
                            ps[:],
                            lhsT=wt[:, kc, :],
                            rhs=xt_chunk[:, kc, :],
                            start=(kc == 0), stop=(kc == DK - 1),
                        )
                    nc.scalar.copy(dst[:, hp, tchunk * 512:(tchunk + 1) * 512], ps[:])

            # V natural for all heads over this chunk: out [t128, 512 ch]
            for tt in range(4):
                ps = qkpsum.tile([128, 512], F32, tag="vps")
                for kc in range(DK):
                    nc.tensor.matmul(
                        ps[:],
                        lhsT=xt_chunk[:, kc, tt * 128:(tt + 1) * 128],
                        rhs=wv_sb[:, kc, :],
                        start=(kc == 0), stop=(kc == DK - 1),
                    )
                kt_idx = tchunk * 4 + tt
                for h in range(HC):
                    nc.vector.tensor_copy(
                        v_sb[:, kt_idx, h, 0:64], ps[:, h * 64:(h + 1) * 64])

    if dbg:
        nc.gpsimd.dma_start(dbg["qt"][:], qt_sb[:])
        nc.gpsimd.dma_start(dbg["kt"][:], kt_sb[:])
        nc.gpsimd.dma_start(dbg["v"][:], v_sb[:])

    # ---- phase C: causal attention -------------------------------------
    with tc.tile_pool(name="spsum", bufs=2, space="PSUM") as spsum, \
         tc.tile_pool(name="opsum", bufs=2, space="PSUM") as opsum, \
         tc.tile_pool(name="exps", bufs=6) as expp, \
         tc.tile_pool(name="norm", bufs=4) as normp:

        for hp in range(HP):
            for qb in range(QB):
                ext = 4 * (qb + 1)          # causal k-tile extent
                o_ps = [opsum.tile([65, 512], F32, tag=f"ops{i}", name=f"ops{i}") for i in (0, 1)]
                for kt in range(ext):
                    e_t = []
                    for i in range(2):      # two heads of the pair
                        s_ps = spsum.tile([128, 512], F32, tag=f"sps{i}")
                        nc.tensor.matmul(
                            s_ps[:],
                            lhsT=kt_sb[64 * i:64 * (i + 1), hp,
                                       kt * 128:(kt + 1) * 128],
                            rhs=qt_sb[64 * i:64 * (i + 1), hp,
                                      qb * 512:(qb + 1) * 512],
                            start=True, stop=True,
                            tile_position=(64 * i, 0),
                        )
                        e = expp.tile([128, 512], FP, tag=f"exp{i}")
                        nc.scalar.activation(
                            e[:], s_ps[:], mybir.ActivationFunctionType.Exp,
                            scale=SCALE)
                        if kt >= 4 * qb:    # diagonal band -> mask
                            nc.vector.tensor_tensor(
                                e[:], e[:], masks[kt - 4 * qb][:].bitcast(FP),
                                mybir.AluOpType.mult)
                        e_t.append(e)
                    for i in range(2):
                        nc.tensor.matmul(
                            o_ps[i][:],
                            lhsT=v_sb[:, kt, 2 * hp + i, :],
                            rhs=e_t[i][:],
                            start=(kt == 0), stop=(kt == ext - 1),
                        )
                # normalize: y = O[0:64] / O[64]  (ones-row sum = softmax denom)
                for i in range(2):
                    rec = normp.tile([1, 512], F32, tag=f"rec{i}")
                    nc.vector.reciprocal(rec[:], o_ps[i][64:65, :])
                    rec_b = normp.tile([64, 512], F32, tag=f"recb{i}", name="rec_b")
                    nc.gpsimd.partition_broadcast(rec_b[:], rec[:])
                    nc.vector.tensor_tensor(
                        y_sb[64 * i:64 * (i + 1), hp, qb * 512:(qb + 1) * 512],
                        o_ps[i][0:64, :],
                        rec_b[:],
                        mybir.AluOpType.mult)

    if dbg:
        nc.gpsimd.dma_start(dbg["y"][:], y_sb[:])

    qkv_ctx.close()

    # ---- phase D: AllGather + projection -------------------------------
    with tc.tile_pool(name="dram", bufs=1, space="DRAM") as dram, \
         tc.tile_pool(name="projw", bufs=1) as projw, \
         tc.tile_pool(name="yg", bufs=1) as ygp, \
         tc.tile_pool(name="ppsum", bufs=4, space="PSUM") as ppsum, \
         tc.tile_pool(name="ostage", bufs=3) as ostage, \
         tc.tile_pool(name="selp", bufs=1) as selp:

        wp_sb = projw.tile([128, DK, D], FP)
        nc.sync.dma_start(wp_sb[:], w_proj.rearrange("(o p) f -> p o f", p=128))

        y_loc = dram.tile([HP, 128, T], FP)
        y_all = dram.tile([N_CORES * HP, 128, T], FP, addr_space="Shared")

        nc.sync.dma_start(y_loc[:].rearrange("c p t -> p c t"), y_sb[:])
        if NO_COLLECTIVE:
            nc.sync.dma_start(y_all[0:HP], y_loc[:])
        else:
            nc.gpsimd.collective_compute(
                "AllGather",
                mybir.AluOpType.bypass,
                ins=[y_loc[:].opt()],
                outs=[y_all[:].opt()],
                replica_groups=[list(range(N_CORES))],
            )

        # dynamic gather of my batch's full y^T [1024 ch, my 1024 tokens]
        sel_sb = selp.tile([1, 2], mybir.dt.uint32)
        nc.sync.dma_start(sel_sb[:], sel[:])
        slot_reg = nc.sync.alloc_register("slot0")
        col_reg = nc.sync.alloc_register("col0")
        nc.sync.reg_load(slot_reg, sel_sb[0:1, 0:1])
        nc.sync.reg_load(col_reg, sel_sb[0:1, 1:2])
        slot0 = nc.sync.snap(slot_reg)
        col0 = nc.sync.snap(col_reg)

        yg = ygp.tile([128, DK, MYT], FP)
        for ct in range(DK):
            nc.sync.dma_start(
                yg[:, ct, :],
                y_all[bass.ds(slot0 * HP + ct, 1), :,
                      bass.ds(col0, MYT)].opt())

        if dbg:
            nc.gpsimd.dma_start(dbg["yg"][:], yg[:])
            nc.gpsimd.dma_start(dbg["yall"][:], y_all[:])
            nc.gpsimd.dma_start(dbg["yloc"][:], y_loc[:])

        for tt in range(MYT // 128):
            for nn in range(2):
                ps = ppsum.tile([128, 512], F32, tag="pps")
                for kc in range(DK):
                    nc.tensor.matmul(
                        ps[:],
                        lhsT=yg[:, kc, tt * 128:(tt + 1) * 128],
                        rhs=wp_sb[:, kc, nn * 512:(nn + 1) * 512],
                        start=(kc == 0), stop=(kc == DK - 1),
                    )
                o_sb = ostage.tile([128, 512], F32, tag="osb")
                nc.scalar.copy(o_sb[:], ps[:])
                nc.sync.dma_start(
                    out[tt * 128:(tt + 1) * 128, nn * 512:(nn + 1) * 512],
                    o_sb[:])

    ctx.close()


_NC_CACHE = None
LAST_RESULT = None


def kernel(x, w_qkv, w_proj):
    global _NC_CACHE, LAST_RESULT
    x = np.asarray(x, dtype=np.float32)
    w_qkv = np.asarray(w_qkv, dtype=np.float32)
    w_proj = np.asarray(w_proj, dtype=np.float32)

    if _NC_CACHE is None:
        _NC_CACHE = build_kernel()
    nc = _NC_CACHE

    in_maps = []
    for m in range(N_CORES):
        b, g = m // 2, m % 2
        w_my = np.concatenate(
            [w_qkv[:, g * 512:(g + 1) * 512],
             w_qkv[:, 1024 + g * 512:1024 + (g + 1) * 512],
             w_qkv[:, 2048 + g * 512:2048 + (g + 1) * 512]], axis=1)
        in_maps.append({
            "x_b": np.ascontiguousarray(x[b]),
            "w_qkv_my": np.ascontiguousarray(w_my),
            "w_proj": w_proj,
            "sel": np.array([[2 * b, g * 1024]], dtype=np.uint32),
        })

    res = run_bass_kernel_spmd(nc, in_maps, core_ids=list(range(N_CORES)))
    LAST_RESULT = res
    out = np.empty((B, T, D), dtype=np.float32)
    for m in range(N_CORES):
        b, g = m // 2, m % 2
        out[b, g * 1024:(g + 1) * 1024, :] = res.results[m]["out"]
    return out


# revision 28
# speedup vs baseline: 1.2982x; 1.2982x over previous
"""Causal self-attention (B=4, T=2048, D=1024, H=16) on 8 Trainium2 cores.

Sharding: core m = (batch b=m//2, head-group g=m%2 of 8 heads).
Each core computes QKV for its 8 heads of its batch, full causal attention
(exact lower-triangular tile skip, identical tile structure on every core),
writes its attention output transposed (y^T, [512 ch, 2048 tok]) to DRAM,
AllGather(8) so every core holds all head-groups, then projects its own
1024-token slice with the full w_proj.  Per-core programs are identical
(SPMD); the only cross-core variation is input data plus two uint32 scalars
driving dynamic DMA offsets in the post-AllGather gather.

All matmul operands are float32r (FP22-read, 1-pass PE) for bf16-class
throughput at near-fp32 accuracy; accumulation is fp32 in PSUM.
"""

import numpy as np

import concourse.bass as bass
import concourse.mybir as mybir
import concourse.tile as tile
from concourse import bacc
from concourse.bass_utils import run_bass_kernel_spmd
from concourse.masks import make_identity

FP = mybir.dt.float32r
F32 = mybir.dt.float32

# Problem constants (per spec; hardcoded).
B, T, D, H = 4, 2048, 1024, 16
DH = 64                      # head dim
N_CORES = 8
HC = H // 2                  # heads per core = 8
HP = HC // 2                 # head pairs per core = 4
DK = D // 128                # model-dim contraction tiles = 8
TT = T // 128                # token tiles of 128 = 16
QB = T // 512                # q blocks of 512 = 4
TC = T // 512                # token chunks of 512 = 4
MYT = T // 2                 # tokens this core projects = 1024
SCALE = 1.0 / 8.0            # 1/sqrt(DH)
DEBUG = False                # add intermediate dumps as extra outputs
NO_COLLECTIVE = False        # replace AllGather with local copy (TimelineSim)


def build_kernel(iters=1):
    nc = bacc.Bacc("TRN2", target_bir_lowering=False, debug=False,
                   num_devices=N_CORES)

    x_bT = nc.dram_tensor("x_bT", [D, T], FP, kind="ExternalInput").ap()
    w_qkv_my = nc.dram_tensor("w_qkv_my", [D, 3 * 512], FP,
                              kind="ExternalInput").ap()
    w_proj = nc.dram_tensor("w_proj", [D, D], FP, kind="ExternalInput").ap()
    sel = nc.dram_tensor("sel", [1, 2], mybir.dt.uint32,
                         kind="ExternalInput").ap()
    hsel = nc.dram_tensor("hsel", [128, 1], F32, kind="ExternalInput").ap()
    out = nc.dram_tensor("out", [MYT, D], F32, kind="ExternalOutput").ap()
    dbg = {}
    if DEBUG:
        dbg["qt"] = nc.dram_tensor("dbg_qt", [128, HP, T], F32,
                                   kind="ExternalOutput").ap()
        dbg["kt"] = nc.dram_tensor("dbg_kt", [128, HP, T], F32,
                                   kind="ExternalOutput").ap()
        dbg["v"] = nc.dram_tensor("dbg_v", [128, TT, HC, 65], F32,
                                  kind="ExternalOutput").ap()
        dbg["y"] = nc.dram_tensor("dbg_y", [128, HP, T], F32,
                                  kind="ExternalOutput").ap()
        dbg["yg"] = nc.dram_tensor("dbg_yg", [128, DK, MYT], F32,
                                   kind="ExternalOutput").ap()

    with tile.TileContext(nc) as tc:
        for _ in range(iters):
            _emit(tc, x_bT, w_qkv_my, w_proj, sel, hsel, out, dbg)

    nc.compile()
    return nc


def _emit(tc, x_bT, w_qkv_my, w_proj, sel, hsel, out, dbg=None):
    from contextlib import ExitStack
    nc = tc.nc
    ctx = ExitStack()

    # ---- constants ------------------------------------------------------
    const = ctx.enter_context(tc.tile_pool(name="const", bufs=1))
    # single diagonal mask [128 k, 128 q-local]: keep (1.0) where kk <= qq
    mask_f32 = const.tile([128, 128], F32)
    nc.gpsimd.memset(mask_f32[:], 1.0)
    nc.gpsimd.affine_select(
        out=mask_f32[:], in_=mask_f32[:],
        compare_op=mybir.AluOpType.is_ge,
        fill=0.0, base=0,
        pattern=[[1, 128]],       # + qq
        channel_multiplier=-1,    # - kk
    )
    mask = const.tile([128, 128], FP)
    nc.vector.tensor_copy(mask[:], mask_f32[:])

    # ---- persistent SBUF ------------------------------------------------
    persist = ctx.enter_context(tc.tile_pool(name="persist", bufs=1))
    y_sb = persist.tile([128, HP, T], FP, tag="y")
    qkv_ctx = ExitStack()
    qkvp = qkv_ctx.enter_context(tc.tile_pool(name="qkvp", bufs=1))
    qt_sb = qkvp.tile([128, HP, T], FP, tag="qt")
    kt_sb = qkvp.tile([128, HP, T], FP, tag="kt")
    v_sb = qkvp.tile([128, TT, HC, 65], FP, tag="v")

    ones_f32 = const.tile([128, TT, HC, 1], F32)
    nc.vector.memset(ones_f32[:], 1.0)
    nc.vector.tensor_copy(v_sb[:, :, :, 64:65], ones_f32[:])

    # DRAM for the gather (created early so AGs can interleave)
    dram_ctx = ExitStack()
    dram = dram_ctx.enter_context(tc.tile_pool(name="dram", bufs=1, space="DRAM"))
    y_loc_a = dram.tile([HP, 128, T // 2], FP)
    y_loc_b = dram.tile([HP, 128, T // 2], FP)
    y_all_a = dram.tile([N_CORES * HP, 128, T // 2], FP, addr_space="Shared")
    y_all_b = dram.tile([N_CORES * HP, 128, T // 2], FP, addr_space="Shared")

    # ---- interleaved qkv production + attention -------------------------
    w_re = w_qkv_my.rearrange("(o p) f -> p o f", p=128)
    x_re = x_bT.rearrange("(o p) t -> p o t", p=128)
    with tc.tile_pool(name="wqkv", bufs=1) as wpool, \
         tc.tile_pool(name="wqk", bufs=2) as wqkp, \
         tc.tile_pool(name="xt", bufs=1) as xtp, \
         tc.tile_pool(name="ps512", bufs=4, space="PSUM") as ps512, \
         tc.tile_pool(name="opsum", bufs=2, space="PSUM") as opsum, \
         tc.tile_pool(name="exps", bufs=6) as expp, \
         tc.tile_pool(name="norm", bufs=2) as normp:

        wv_sb = wpool.tile([128, DK, 512], FP)
        nc.sync.dma_start(wv_sb[:], w_re[:, :, 1024:1536])

        for j in range(TC):          # token chunk j == q-block j
            # -- x^T chunk j: direct DMA (x pre-transposed on host) --
            xt_chunk = xtp.tile([128, DK, 512], FP, tag="xtc")
            for kc in range(DK):
                nc.sync.dma_start(xt_chunk[:, kc, :],
                                  x_re[:, kc, j * 512:(j + 1) * 512])

            # -- Q^T, K^T chunk j for each head pair --
            for hp in range(HP):
                for which, dst in ((0, qt_sb), (1, kt_sb)):
                    wt = wqkp.tile([128, DK, 128], FP, tag="wqk", name="wt")
                    nc.sync.dma_start(
                        wt[:], w_re[:, :, which * 512 + hp * 128:
                                    which * 512 + (hp + 1) * 128])
                    ps = ps512.tile([128, 512], F32, tag="ps512", name="qkps")
                    for kc in range(DK):
                        nc.tensor.matmul(
                            ps[:], lhsT=wt[:, kc, :], rhs=xt_chunk[:, kc, :],
                            start=(kc == 0), stop=(kc == DK - 1))
                    nc.scalar.copy(dst[:, hp, j * 512:(j + 1) * 512], ps[:])

            # -- V chunk j (all heads) --
            for tt in range(4):
                ps = ps512.tile([128, 512], F32, tag="ps512", name="vps")
                for kc in range(DK):
                    nc.tensor.matmul(
                        ps[:], lhsT=xt_chunk[:, kc, tt * 128:(tt + 1) * 128],
                        rhs=wv_sb[:, kc, :],
                        start=(kc == 0), stop=(kc == DK - 1))
                kt_idx = j * 4 + tt
                for h in range(HC):
                    nc.vector.tensor_copy(
                        v_sb[:, kt_idx, h, 0:64], ps[:, h * 64:(h + 1) * 64])

            # -- attention for q-block j (all head pairs) --
            qb = j
            ext = 4 * (qb + 1)
            for hp in range(HP):
                o_ps = [opsum.tile([65, 512], F32, tag=f"ops{i}", name=f"ops{i}")
                        for i in (0, 1)]
                for kt in range(ext):
                    dk = kt - 4 * qb          # >=0 on diagonal band
                    o = 128 * dk if dk >= 0 else 0
                    for i in range(2):
                        s_ps = ps512.tile([128, 512], F32, tag="ps512",
                                          name=f"sps{i}")
                        nc.tensor.matmul(
                            s_ps[:, o:512],
                            lhsT=kt_sb[64 * i:64 * (i + 1), hp,
                                       kt * 128:(kt + 1) * 128],
                            rhs=qt_sb[64 * i:64 * (i + 1), hp,
                                      qb * 512 + o:(qb + 1) * 512],
                            start=True, stop=True,
                            tile_position=(64 * i, 0))
                        e = expp.tile([128, 512], FP, tag=f"exp{i}", name=f"e{i}")
                        nc.scalar.activation(
                            e[:, o:512], s_ps[:, o:512],
                            mybir.ActivationFunctionType.Exp, scale=SCALE)
                        if dk >= 0:
                            nc.vector.tensor_tensor(
                                e[:, o:o + 128], e[:, o:o + 128], mask[:],
                                mybir.AluOpType.mult)
                        nc.tensor.matmul(
                            o_ps[i][:, o:512],
                            lhsT=v_sb[:, kt, 2 * hp + i, :],
                            rhs=e[:, o:512],
                            start=(kt == 0), stop=(kt == ext - 1))
                for i in range(2):
                    rec = normp.tile([1, 512], F32, tag="rec", name="rec")
                    nc.vector.reciprocal(rec[:], o_ps[i][64:65, :])
                    rec_b = normp.tile([64, 512], F32, tag="recb",
                                       name="rec_b")
                    nc.gpsimd.partition_broadcast(rec_b[:], rec[:])
                    nc.vector.tensor_tensor(
                        y_sb[64 * i:64 * (i + 1), hp,
                             qb * 512:(qb + 1) * 512],
                        o_ps[i][0:64, :], rec_b[:], mybir.AluOpType.mult)

            # -- after qb=1: first-half y is complete -> overlap AG #1 --
            if j == 1:
                nc.sync.dma_start(
                    y_loc_a[:].rearrange("c p t -> p c t"),
                    y_sb[:, :, 0:T // 2])
                if NO_COLLECTIVE:
                    nc.sync.dma_start(y_all_a[0:HP], y_loc_a[:])
                else:
                    nc.gpsimd.collective_compute(
                        "AllGather", mybir.AluOpType.bypass,
                        ins=[y_loc_a[:].opt()],
                        outs=[y_all_a[:].opt()],
                        replica_groups=[list(range(N_CORES))])

    if dbg:
        nc.gpsimd.dma_start(dbg["qt"][:], qt_sb[:])
        nc.gpsimd.dma_start(dbg["kt"][:], kt_sb[:])
        nc.gpsimd.dma_start(dbg["v"][:], v_sb[:])
        nc.gpsimd.dma_start(dbg["y"][:], y_sb[:])

    qkv_ctx.close()

    # ---- AllGather #2 + projection --------------------------------------
    with tc.tile_pool(name="projw", bufs=1) as projw, \
         tc.tile_pool(name="yg", bufs=1) as ygp, \
         tc.tile_pool(name="ppsum", bufs=4, space="PSUM") as ppsum, \
         tc.tile_pool(name="ostage", bufs=3) as ostage, \
         tc.tile_pool(name="selp", bufs=1) as selp:

        wp_sb = projw.tile([128, DK, D], FP)
        nc.sync.dma_start(wp_sb[:], w_proj.rearrange("(o p) f -> p o f", p=128))

        nc.sync.dma_start(
            y_loc_b[:].rearrange("c p t -> p c t"),
            y_sb[:, :, T // 2:T])
        if NO_COLLECTIVE:
            nc.sync.dma_start(y_all_b[0:HP], y_loc_b[:])
        else:
            nc.gpsimd.collective_compute(
                "AllGather", mybir.AluOpType.bypass,
                ins=[y_loc_b[:].opt()],
                outs=[y_all_b[:].opt()],
                replica_groups=[list(range(N_CORES))])

        sel_sb = selp.tile([1, 2], mybir.dt.uint32)
        nc.sync.dma_start(sel_sb[:], sel[:])
        idx_reg = nc.sync.alloc_register("idx0")
        nc.sync.reg_load(idx_reg, sel_sb[0:1, 0:1])
        idx0 = nc.sync.snap(idx_reg)

        hs_sb = selp.tile([128, 1], F32, name="hs_sb")
        nc.sync.dma_start(hs_sb[:], hsel[:])

        yg = ygp.tile([128, DK, MYT], FP)
        ga = ygp.tile([128, DK, MYT], FP, name="ga")
        gb = ygp.tile([128, DK, MYT], FP, name="gb")
        for ct in range(DK):
            nc.sync.dma_start(
                ga[:, ct, :], y_all_a[bass.ds(idx0 + ct, 1), :, :].opt())
            nc.sync.dma_start(
                gb[:, ct, :], y_all_b[bass.ds(idx0 + ct, 1), :, :].opt())
            # yg = gb + (ga - gb) * hsel   (hsel = 1.0 on first-half cores)
            nc.vector.tensor_tensor(
                ga[:, ct, :], ga[:, ct, :], gb[:, ct, :], mybir.AluOpType.subtract)
            nc.vector.scalar_tensor_tensor(
                yg[:, ct, :], ga[:, ct, :], hs_sb[:], gb[:, ct, :],
                mybir.AluOpType.mult, mybir.AluOpType.add)

        if dbg:
            nc.gpsimd.dma_start(dbg["yg"][:], yg[:])

        for tt in range(MYT // 128):
            for nn in range(2):
                ps = ppsum.tile([128, 512], F32, tag="pps")
                for kc in range(DK):
                    nc.tensor.matmul(
                        ps[:], lhsT=yg[:, kc, tt * 128:(tt + 1) * 128],
                        rhs=wp_sb[:, kc, nn * 512:(nn + 1) * 512],
                        start=(kc == 0), stop=(kc == DK - 1))
                o_sb = ostage.tile([128, 512], F32, tag="osb")
                nc.vector.tensor_copy(o_sb[:], ps[:])
                nc.sync.dma_start(
                    out[tt * 128:(tt + 1) * 128, nn * 512:(nn + 1) * 512],
                    o_sb[:])

    dram_ctx.close()
    ctx.close()


_NC_CACHE = None
LAST_RESULT = None


def kernel(x, w_qkv, w_proj):
    global _NC_CACHE, LAST_RESULT
    x = np.asarray(x, dtype=np.float32)
    w_qkv = np.asarray(w_qkv, dtype=np.float32)
    w_proj = np.asarray(w_proj, dtype=np.float32)

    if _NC_CACHE is None:
        _NC_CACHE = build_kernel()
    nc = _NC_CACHE

    in_maps = []
    for m in range(N_CORES):
        b, g = m // 2, m % 2
        w_my = np.concatenate(
            [w_qkv[:, g * 512:(g + 1) * 512],
             w_qkv[:, 1024 + g * 512:1024 + (g + 1) * 512],
             w_qkv[:, 2048 + g * 512:2048 + (g + 1) * 512]], axis=1)
        in_maps.append({
            "x_bT": np.ascontiguousarray(x[b].T),
            "w_qkv_my": np.ascontiguousarray(w_my),
            "w_proj": w_proj,
            "sel": np.array([[8 * b, 0]], dtype=np.uint32),
            "hsel": np.full((128, 1), 1.0 - g, dtype=np.float32),
        })

    res = run_bass_kernel_spmd(nc, in_maps, core_ids=list(range(N_CORES)))
    LAST_RESULT = res
    out = np.empty((B, T, D), dtype=np.float32)
    for m in range(N_CORES):
        b, g = m // 2, m % 2
        out[b, g * 1024:(g + 1) * 1024, :] = res.results[m]["out"]
    return out
